# revision 1
# baseline (speedup 1.0000x reference)
# MoE routing + sparse-frequency inverse FFT2 kernel for Trainium2 (8 NeuronCores).
#
# Math: out_b = ALPHA * Re(ifft2(mask_b)) where mask_b has 4096 nonzero
# frequencies (top-2 experts x 2048 each).  With the symmetric real DFT basis
#   C[x,u] = cos(2*pi*x*u/768)/768,  S[x,u] = sin(2*pi*x*u/768)/768
# the dense iFFT2 factorizes into four 768^3 matmuls per sample:
#   out = (C @ (M @ C) - S @ (M @ S)) * ALPHA
# Device work per core (4 samples): router GEMM, top-2 selection and weights,
# per-expert entry gather (row-granular indirect DMA at offsets computed from
# the one-hot expert selection), sparse->dense mask build via iota/compare
# one-hots placed with PE matmuls, then the four big matmuls in float32r.
# Host only prepares input-layout constants: the C/S tables and a bucketed,
# padded, partition-major re-layout of the static (list_indices, coeff)
# tables, plus batch sharding.
#
# Element-granular DMA scatter is avoided on purpose: TRN2's indirect DMA is
# row-granular (one offset per partition, contiguous run per partition), so
# the mask is built from gathered (u, v, val) entry groups instead.

import sys

sys.path.insert(0, "/opt/trn_rl_repo")

import numpy as np

import concourse.bacc as bacc
import concourse.mybir as mybir
import concourse.tile as tile
from concourse.bass import IndirectOffsetOnAxis
from concourse.bass_utils import run_bass_kernel_spmd
from concourse.masks import make_identity

N = 768
E = 64
NF = 2048
B = 32
NCORES = 8
BPC = B // NCORES          # samples per core
NBLK = 6                   # 768 / 128
ALPHA = 300.0
GRID = N * N
HALF = N // 2 + 2          # 386 computed stage-1 columns (even width for f32r)

# per-(expert, v-chunk) buckets, sub-bucketed by u-range so each build matmul
# is one bank-aligned instruction: u in [0,512) padded to 384, u in [512,768)
# padded to 256.  Pads sit >=11 sigma above the expected bucket fills.
HB = ((0, 512, 384), (512, 256, 256))   # (u0, uwidth, pad)
BROW = sum(p for _, _, p in HB)          # 640 entries per (expert, v-chunk)
EROW = NBLK * BROW                       # 3840 entries per expert
COLS = EROW // 128                       # 30 gather columns per expert slot

F32 = mybir.dt.float32
F32R = mybir.dt.float32r
I32 = mybir.dt.int32
AOT = mybir.AluOpType

KERNEL_TRACE = False       # test harness can flip this to profile
LAST_RESULT = None

_NC = None


def _build():
    nc = bacc.Bacc(trn_type="TRN2")

    cls4 = nc.dram_tensor("cls4", [BPC, N], F32, kind="ExternalInput")
    wr = nc.dram_tensor("wr", [E, N], F32, kind="ExternalInput")
    br = nc.dram_tensor("br", [E], F32, kind="ExternalInput")
    u2 = nc.dram_tensor("u2", [E, EROW], F32, kind="ExternalInput")
    vm2 = nc.dram_tensor("vm2", [E, EROW], F32, kind="ExternalInput")
    cv2 = nc.dram_tensor("cv2", [E, EROW], F32, kind="ExternalInput")
    bases = nc.dram_tensor("bases", [E, 1], F32, kind="ExternalInput")
    jm = nc.dram_tensor("jm", [128, 128], F32R, kind="ExternalInput")
    ct = nc.dram_tensor("ct", [N, N], F32R, kind="ExternalInput")
    st = nc.dram_tensor("st", [N, N], F32R, kind="ExternalInput")
    out4 = nc.dram_tensor("out4", [BPC, N, N], F32, kind="ExternalOutput")

    with tile.TileContext(nc) as tc:
        with (
            tc.tile_pool(name="const", bufs=1) as cpool,
            tc.tile_pool(name="tables", bufs=1) as tpool,
            tc.tile_pool(name="routing", bufs=1) as rpool,
            tc.tile_pool(name="gath", bufs=1) as gpool,
            tc.tile_pool(name="build", bufs=20) as bpool,
            tc.tile_pool(name="mt", bufs=2) as mtpool,
            tc.tile_pool(name="pq", bufs=1) as pqpool,
            tc.tile_pool(name="outp", bufs=3) as opool,
            tc.tile_pool(name="psA", bufs=2, space="PSUM") as psA,
            tc.tile_pool(name="psA1", bufs=3, space="PSUM") as psA1,
            tc.tile_pool(name="psB", bufs=1, space="PSUM") as psB,
            tc.tile_pool(name="mir", bufs=2) as mirpool,
            tc.tile_pool(name="t1p", bufs=2) as t1pool,
        ):
            ident = cpool.tile([128, 128], F32)
            make_identity(nc, ident[:])
            ones1 = cpool.tile([1, 128], F32)
            nc.vector.memset(ones1[:], 1.0)
            ones14 = cpool.tile([1, BPC], F32)
            nc.vector.memset(ones14[:], 1.0)
            i768 = cpool.tile([128, N], I32)
            nc.gpsimd.iota(i768[:], pattern=[[1, N]], base=0, channel_multiplier=0)
            i768f = cpool.tile([128, N], F32)
            nc.vector.tensor_copy(i768f[:], i768[:])
            i128f = cpool.tile([128, 128], F32)
            nc.vector.tensor_copy(i128f[:], i768[:, 0:128])
            io24 = cpool.tile([128, 1], I32)
            nc.gpsimd.iota(io24[:], pattern=[[0, 1]], base=0, channel_multiplier=COLS)
            io24f = cpool.tile([128, 1], F32)
            nc.vector.tensor_copy(io24f[:], io24[:])

            br_sb = rpool.tile([1, E], F32)
            nc.sync.dma_start(out=br_sb[:], in_=br[None, :])
            bases_sb = rpool.tile([E, 1], F32)
            nc.sync.dma_start(out=bases_sb[:], in_=bases[:])
            jJ = cpool.tile([128, 128], F32R)
            nc.sync.dma_start(out=jJ[:], in_=jm[:])

            # ---- router: logits = cls4 @ Wr.T + br ----
            comb = rpool.tile([E + BPC, N], F32)
            nc.sync.dma_start(out=comb[0:BPC, :], in_=cls4[:])
            nc.sync.dma_start(out=comb[BPC : BPC + E, :], in_=wr[:])
            combt = rpool.tile([128, NBLK * (E + BPC)], F32)
            for j in range(NBLK):
                tp = psB.tile([128, E + BPC], F32, tag="small")
                nc.tensor.transpose(
                    tp[:],
                    comb[0 : E + BPC, 128 * j : 128 * (j + 1)],
                    ident[0 : E + BPC, 0 : E + BPC],
                )
                nc.scalar.copy(combt[:, (E + BPC) * j : (E + BPC) * (j + 1)], tp[:])
            lg_ps = psB.tile([BPC, E], F32, tag="small")
            for j in range(NBLK):
                base = (E + BPC) * j
                nc.tensor.matmul(
                    lg_ps[:],
                    lhsT=combt[:, base : base + BPC],
                    rhs=combt[:, base + BPC : base + BPC + E],
                    start=(j == 0),
                    stop=False,
                )
            nc.tensor.matmul(
                lg_ps[:], lhsT=ones14[:], rhs=br_sb[:], start=False, stop=True
            )
            logits = rpool.tile([BPC, E], F32)
            nc.vector.tensor_copy(logits[:], lg_ps[:])

            # ---- top-2, renormalized weights, one-hot selectors ----
            max8 = rpool.tile([BPC, 8], F32)
            nc.vector.max(out=max8[:], in_=logits[:])
            l0 = max8[:, 0:1]
            l1 = max8[:, 1:2]
            d = rpool.tile([BPC, 1], F32)
            nc.vector.tensor_sub(d[:], l1, l0)  # l1 - l0
            dT_ps = psB.tile([1, BPC], F32, tag="small")
            nc.tensor.transpose(dT_ps[:], d[:], ident[0:BPC, 0:BPC])
            dT = rpool.tile([1, BPC], F32)
            nc.vector.tensor_copy(dT[:], dT_ps[:])
            w1T = rpool.tile([1, BPC], F32)
            nc.scalar.activation(w1T[:], dT[:], mybir.ActivationFunctionType.Sigmoid)
            w0T = rpool.tile([1, BPC], F32)
            nc.scalar.activation(
                w0T[:], dT[:], mybir.ActivationFunctionType.Sigmoid, scale=-1.0
            )
            oh1 = rpool.tile([BPC, E], F32)
            oh2 = rpool.tile([BPC, E], F32)
            nc.vector.tensor_scalar(oh1[:], logits[:], l0, None, op0=AOT.is_equal)
            nc.vector.tensor_scalar(oh2[:], logits[:], l1, None, op0=AOT.is_equal)
            selT = []
            for srcap in (oh1, oh2):
                sp = psB.tile([E, BPC], F32, tag="small")
                nc.tensor.transpose(sp[:], srcap[:], ident[0:BPC, 0:BPC])
                sbt = rpool.tile([E, BPC], F32, tag=f"selT{len(selT)}")
                nc.vector.tensor_copy(sbt[:], sp[:])
                selT.append(sbt)
            o1T, o2T = selT

            # per-sample scalar rows [1, BPC]: expert table offsets
            eT = []
            for oT in (o1T, o2T):
                ep = psB.tile([1, BPC], F32, tag="small")
                nc.tensor.matmul(
                    ep[:], lhsT=bases_sb[:], rhs=oT[:], start=True, stop=True
                )
                es = rpool.tile([1, BPC], F32, tag=f"eT{len(eT)}")
                nc.vector.tensor_copy(es[:], ep[:])
                eT.append(es)

            # broadcast each scalar row to all 128 partitions: [128, BPC]
            bc = []
            for rowap in (eT[0], eT[1], w0T, w1T):
                bp = psB.tile([128, BPC], F32, tag="small")
                nc.tensor.matmul(
                    bp[:], lhsT=ones1[:], rhs=rowap[:], start=True, stop=True
                )
                bs = rpool.tile([128, BPC], F32, tag=f"bc{len(bc)}")
                nc.vector.tensor_copy(bs[:], bp[:])
                bc.append(bs)
            ebc = bc[0:2]    # expert base offsets per slot
            wbc = bc[2:4]    # expert weights per slot

            # ---- C/S table loads AFTER the routing-phase emission so the
            # small router DMAs aren't queued behind 4.7 MB on the sync FIFO
            ct_sb = tpool.tile([128, NBLK * N], F32R, tag="ct")
            st_sb = tpool.tile([128, NBLK * N], F32R, tag="st")
            for j in range(NBLK):
                nc.sync.dma_start(
                    out=ct_sb[:, N * j : N * (j + 1)],
                    in_=ct[128 * j : 128 * (j + 1), :],
                )
                nc.sync.dma_start(
                    out=st_sb[:, N * j : N * (j + 1)],
                    in_=st[128 * j : 128 * (j + 1), :],
                )

            ych = [(0, 512), (512, 256)]

            def emit_d(b, di, mc):
                dps = psA.tile([128, N], F32, tag="mm")
                for c0, cw in ych:
                    nc.tensor.matmul(
                        dps[:, c0 : c0 + cw],
                        lhsT=jJ[:],
                        rhs=mc[:, c0 : c0 + cw],
                        start=True, stop=True,
                    )
                ob = opool.tile([128, N], F32, tag="ob")
                nc.scalar.copy(ob[:], dps[:])
                nc.scalar.dma_start(
                    out=out4[:][b][128 * (4 + di) : 128 * (5 + di), :], in_=ob[:]
                )

            # ---- gather ALL samples' (u, vm, coeff) entry tables upfront ----
            allg = []
            for b in range(BPC):
                gus, gvms, gcws = [], [], []
                for slot in range(2):
                    offf = gpool.tile([128, 1], F32, tag="offf")
                    nc.vector.tensor_add(offf[:], ebc[slot][:, b : b + 1], io24f[:])
                    offs = gpool.tile([128, 1], I32, tag="offs")
                    nc.vector.tensor_copy(offs[:], offf[:])
                    gu = gpool.tile([128, COLS], F32, tag=f"gu{b}_{slot}")
                    gvm = gpool.tile([128, COLS], F32, tag=f"gvm{b}_{slot}")
                    gcv = gpool.tile([128, COLS], F32, tag=f"gcv{b}_{slot}")
                    for tab, dst in ((u2, gu), (vm2, gvm), (cv2, gcv)):
                        nc.gpsimd.indirect_dma_start(
                            out=dst[:],
                            out_offset=None,
                            in_=tab[:],
                            in_offset=IndirectOffsetOnAxis(ap=offs[:], axis=1),
                        )
                    gcw = gpool.tile([128, COLS], F32, tag=f"gcw{b}_{slot}")
                    nc.vector.tensor_scalar(
                        gcw[:], gcv[:], wbc[slot][:, b : b + 1], None, op0=AOT.mult
                    )
                    gus.append(gu)
                    gvms.append(gvm)
                    gcws.append(gcw)
                allg.append((gus, gvms, gcws))

            for b in range(BPC):
                gus, gvms, gcws = allg[b]
                # ---- build MT (transposed mask) chunk by chunk on PE ----
                mt_sb = mtpool.tile([128, NBLK * N], F32R, tag="mt")
                for j in range(NBLK):
                    mtps = psA.tile([128, N], F32, tag="mm")
                    colbase = COLS * j // NBLK * 0  # columns laid out per j below
                    for hi, (u0, uw, pad) in enumerate(HB):
                        ng = pad // 128
                        coff = 5 * j + (0 if hi == 0 else HB[0][2] // 128)
                        for slot in range(2):
                            for g in range(ng):
                                col = coff + g
                                voh = bpool.tile([128, 128], F32R, tag="voh")
                                nc.vector.tensor_scalar(
                                    voh[:], i128f[:], gvms[slot][:, col : col + 1],
                                    None, op0=AOT.is_equal,
                                )
                                rhsb = bpool.tile([128, 512], F32R, tag="rhsb")
                                nc.vector.tensor_scalar(
                                    rhsb[:, 0:uw], i768f[:, u0 : u0 + uw],
                                    gus[slot][:, col : col + 1],
                                    gcws[slot][:, col : col + 1],
                                    op0=AOT.is_equal, op1=AOT.mult,
                                )
                                nc.tensor.matmul(
                                    mtps[:, u0 : u0 + uw],
                                    lhsT=voh[:],
                                    rhs=rhsb[:, 0:uw],
                                    start=(slot == 0 and g == 0),
                                    stop=(slot == 1 and g == ng - 1),
                                )
                    nc.scalar.copy(mt_sb[:, N * j : N * (j + 1)], mtps[:])

                # ---- stage 1 (paired): P = 300*(M @ C), Qn = -300*(M @ S) ----
                # only columns [0, HALF) are computed; C-column symmetry gives
                # P[:, N-y] = P[:, y] and Qn[:, N-y] = -Qn[:, y].
                p_sb = pqpool.tile([128, NBLK * N], F32R, tag="p")
                q_sb = pqpool.tile([128, NBLK * N], F32R, tag="q")
                for i in range(NBLK):
                    pps = psA1.tile([128, HALF], F32, tag="mm1")
                    qps = psA1.tile([128, HALF], F32, tag="mm1")
                    for k in range(NBLK):
                        lhs = mt_sb[:, N * k + 128 * i : N * k + 128 * (i + 1)]
                        nc.tensor.matmul(
                            pps[:], lhsT=lhs, rhs=ct_sb[:, N * k : N * k + HALF],
                            start=(k == 0), stop=(k == NBLK - 1),
                        )
                        nc.tensor.matmul(
                            qps[:], lhsT=lhs, rhs=st_sb[:, N * k : N * k + HALF],
                            start=(k == 0), stop=(k == NBLK - 1),
                        )
                    nc.scalar.mul(p_sb[:, N * i : N * i + HALF], pps[:], ALPHA)
                    nc.scalar.mul(q_sb[:, N * i : N * i + HALF], qps[:], -ALPHA)
                    nc.scalar.copy(
                        p_sb[:, N * i + HALF : N * (i + 1)],
                        p_sb[:][:, N * i + (N - HALF) : N * i : -1],
                    )
                    nc.scalar.mul(
                        q_sb[:, N * i + HALF : N * (i + 1)],
                        q_sb[:][:, N * i + (N - HALF) : N * i : -1],
                        -1.0,
                    )

                # ---- stage 2: rows 0..511 as T1+T2; rows 512..767 mirrored ----
                # T1 = C @ P, T2 = S @ Qn (both already x300).  Row symmetry:
                # out[N-x] = T1[x] - T2[x], realized with shifted anti-identity
                # matmuls (jA, jB) on M_i = T1_i - T2_i.
                mirs = []
                for i in range(4):
                    t1 = psA.tile([128, N], F32, tag="mm")
                    t2 = psA.tile([128, N], F32, tag="mm")
                    for dst, tbl, srcm in ((t1, ct_sb, p_sb), (t2, st_sb, q_sb)):
                        for k in range(NBLK):
                            for c0, cw in ych:
                                nc.tensor.matmul(
                                    dst[:, c0 : c0 + cw],
                                    lhsT=tbl[:, N * k + 128 * i : N * k + 128 * (i + 1)],
                                    rhs=srcm[:, N * k + c0 : N * k + c0 + cw],
                                    start=(k == 0),
                                    stop=(k == NBLK - 1),
                                )
                    t1s = t1pool.tile([128, N], F32, tag="t1")
                    nc.scalar.copy(t1s[:], t1[:])
                    ob = opool.tile([128, N], F32, tag="ob")
                    nc.vector.tensor_tensor(ob[:], t1s[:], t2[:], op=AOT.add)
                    nc.scalar.dma_start(
                        out=out4[:][b][128 * i : 128 * (i + 1), :], in_=ob[:]
                    )
                    # mirror source tiles: mc[d] rows = T1-T2 at x = (2-d)*128 - m
                    if i == 0:
                        m = mirpool.tile([128, N], F32R, tag="mc1")
                        nc.vector.tensor_tensor(m[:], t1s[:], t2[:], op=AOT.subtract)
                        mirs.append(m)  # mc2 body (block 0), row 0 patched later
                    elif i == 1:
                        m = mirpool.tile([128, N], F32R, tag="mc0")
                        nc.vector.tensor_tensor(m[:], t1s[:], t2[:], op=AOT.subtract)
                        mirs.append(m)  # mc1 body (block 1), row 0 patched later
                        nc.vector.tensor_tensor(
                            mirs[0][0:1, :], t1s[0:1, :], t2[0:1, :], op=AOT.subtract
                        )  # mc2 row 0 = block-1 row 0 (x = 128)
                    elif i == 2:
                        nc.vector.tensor_tensor(
                            mirs[1][0:1, :], t1s[0:1, :], t2[0:1, :], op=AOT.subtract
                        )  # mc1 row 0 = block-2 row 0 (x = 256)
                emit_d(b, 0, mirs[1])
                emit_d(b, 1, mirs[0])

    nc.compile()
    return nc


def _get_nc():
    global _NC
    if _NC is None:
        _NC = _build()
    return _NC


def _host_tables():
    a = np.arange(N, dtype=np.int64)
    ang = (2.0 * np.pi / N) * ((a[:, None] * a[None, :]) % N)
    ctv = (np.cos(ang) / N).astype(np.float32)
    stv = (np.sin(ang) / N).astype(np.float32)
    return ctv, stv


def _host_entry_tables(list_indices, coeff):
    """Bucket each expert's (u, v, coeff) entries by v-chunk, pad buckets to
    PAD, and lay out partition-major (entry 128*g + p lands at column g of
    partition p's contiguous gather run)."""
    li = list_indices.astype(np.int64)
    uu = li // N
    vv = li % N
    u2 = np.zeros((E, EROW), np.float32)
    vm2 = np.full((E, EROW), -9.0, np.float32)
    cv2 = np.zeros((E, EROW), np.float32)
    for e in range(E):
        for j in range(NBLK):
            selj = vv[e] // 128 == j
            base = BROW * j
            for u0, uw, pad in HB:
                sel = np.where(selj & (uu[e] >= u0) & (uu[e] < u0 + uw))[0]
                cnt = len(sel)
                assert cnt <= pad, f"bucket overflow: e{e} j{j} u{u0}: {cnt}"
                u2[e, base : base + cnt] = uu[e, sel]
                vm2[e, base : base + cnt] = vv[e, sel] - 128 * j
                cv2[e, base : base + cnt] = coeff[e, sel]
                base += pad
    # partition-major runs: table[e, p*COLS + g] = arr[e, 128*g + p]
    perm = np.array([128 * g + p for p in range(128) for g in range(COLS)])
    return u2[:, perm], vm2[:, perm], cv2[:, perm]


def kernel(cls_token, W_router, b_router, coeff, list_indices):
    global LAST_RESULT
    cls_token = np.asarray(cls_token)
    W_router = np.asarray(W_router)
    b_router = np.asarray(b_router)
    coeff = np.asarray(coeff)
    list_indices = np.asarray(list_indices)
    assert cls_token.shape == (B, N) and coeff.shape == (E, NF)
    nc = _get_nc()
    ctv, stv = _host_tables()
    u2v, vm2v, cv2v = _host_entry_tables(list_indices, coeff)
    basesv = (np.arange(E, dtype=np.float32) * EROW).reshape(E, 1)
    jmv = np.zeros((128, 128), np.float32)
    for m_ in range(128):
        jmv[(128 - m_) % 128, m_] = 1.0
    wrr = np.ascontiguousarray(W_router, dtype=np.float32)
    brr = np.ascontiguousarray(b_router, dtype=np.float32)
    in_maps = []
    for c in range(NCORES):
        in_maps.append(
            {
                "cls4": np.ascontiguousarray(
                    cls_token[BPC * c : BPC * (c + 1)], dtype=np.float32
                ),
                "wr": wrr,
                "br": brr,
                "u2": u2v,
                "vm2": vm2v,
                "cv2": cv2v,
                "bases": basesv,
                "jm": jmv,
                "ct": ctv,
                "st": stv,
            }
        )
    res = run_bass_kernel_spmd(
        nc, in_maps, core_ids=list(range(NCORES)), trace=KERNEL_TRACE
    )
    LAST_RESULT = res
    out = np.concatenate([res.results[c]["out4"] for c in range(NCORES)], axis=0)
    return out



# revision 2
# speedup vs baseline: 1.0368x; 1.0368x over previous
# MoE routing + sparse-frequency inverse FFT2 kernel for Trainium2 (8 NeuronCores).
#
# Math: out_b = ALPHA * Re(ifft2(mask_b)), mask_b has 4096 nonzeros (top-2
# experts x 2048).  out = C M C - S M S with C/S the symmetric cos/sin DFT
# tables.  Device pipeline per sample (4 samples per core):
#   1. router GEMM + top-2 + renormalized weights (PE/DVE, tiny)
#   2. gather the sample's (u-index, coeff) entry lists, bucketed by exact v,
#      with one indirect DMA per (slot, table); scale coeff by routing weight
#   3. GPSIMD local_scatter builds the transposed mask MT[v,u] in six
#      [128,768] fp16 chunks (list_indices is a permutation => no dup indices)
#   4. parity fold: cos((v+384)y) = (-1)^y cos(vy) => M+/- = chunk c +- c+3,
#      halving the stage-1 contraction; same fold on u halves stage-2
#   5. stage 1: P,Q packed as [a*C | -a*S] rhs tables for even/odd y (36
#      fp16 matmuls), stage 2: 48 matmuls of 193 columns (y in [0,386) only;
#      out[x, N-y] = T1 - T2 supplies the rest), direct strided DVE assembly
#   6. rows 512..767 via anti-identity PE rotations of sum/dif-swapped bodies

import sys

sys.path.insert(0, "/opt/trn_rl_repo")

import numpy as np

import concourse.bacc as bacc
import concourse.mybir as mybir
import concourse.tile as tile
from concourse.bass import IndirectOffsetOnAxis
from concourse.bass_utils import run_bass_kernel_spmd
from concourse.masks import make_identity

N = 768
E = 64
NF = 2048
B = 32
NCORES = 8
BPC = B // NCORES          # samples per core
ALPHA = 300.0
H = 386                    # computed y-columns [0, H)
HP = 193                   # per-parity column count
NI = 16                    # padded entries per (expert, v) cell (data max: 14)
GW = 6 * NI                # gathered run per partition
CW = 2 * GW                # combined (both slots) columns per partition

F32 = mybir.dt.float32
F32R = mybir.dt.float32r
F16 = mybir.dt.float16
I16 = mybir.dt.int16
I32 = mybir.dt.int32
AOT = mybir.AluOpType

KERNEL_TRACE = False       # test harness can flip this to profile
DEBUG = False              # dump sample-0 intermediates
CONTIG_TEST = False        # debug: contiguous (slot-major) interleave copies
STOP_EARLY = False         # debug: emit only gather+interleave for sample 0
DBG_B = 0                  # which sample debug dumps capture
LAST_RESULT = None

_NC = None


def _build():
    nc = bacc.Bacc(trn_type="TRN2")

    cls4 = nc.dram_tensor("cls4", [BPC, N], F32, kind="ExternalInput")
    wr = nc.dram_tensor("wr", [E, N], F32, kind="ExternalInput")
    br = nc.dram_tensor("br", [E], F32, kind="ExternalInput")
    uidx = nc.dram_tensor("uidx", [E, 128 * GW], F32, kind="ExternalInput")
    cval = nc.dram_tensor("cval", [E, 128 * GW], F32, kind="ExternalInput")
    bases = nc.dram_tensor("bases", [E, 1], F32, kind="ExternalInput")
    jm1 = nc.dram_tensor("jm1", [128, 128], F32R, kind="ExternalInput")
    jm2 = nc.dram_tensor("jm2", [128, 128], F32R, kind="ExternalInput")
    rhse = nc.dram_tensor("rhse", [128, 3 * H], F32, kind="ExternalInput")
    rhso = nc.dram_tensor("rhso", [128, 3 * H], F32, kind="ExternalInput")
    cxa = nc.dram_tensor("cxa", [128, 3 * 256], F32, kind="ExternalInput")
    cxb = nc.dram_tensor("cxb", [128, 3 * 256], F32, kind="ExternalInput")
    sxa = nc.dram_tensor("sxa", [128, 3 * 256], F32, kind="ExternalInput")
    sxb = nc.dram_tensor("sxb", [128, 3 * 256], F32, kind="ExternalInput")
    out4 = nc.dram_tensor("out4", [BPC, N, N], F32, kind="ExternalOutput")
    dbg = {}
    if DEBUG:
        dbg["mtU"] = nc.dram_tensor("dbg_mtU", [128, 6 * N], F16, kind="ExternalOutput")
        dbg["combi"] = nc.dram_tensor("dbg_combi", [128, CW], I16, kind="ExternalOutput")
        dbg["combv"] = nc.dram_tensor("dbg_combv", [128, CW], F16, kind="ExternalOutput")
        dbg["mP"] = nc.dram_tensor("dbg_mP", [128, 3 * N], F16, kind="ExternalOutput")
        dbg["esb"] = nc.dram_tensor("dbg_esb", [128, 6 * H], F16, kind="ExternalOutput")
        dbg["osb"] = nc.dram_tensor("dbg_osb", [128, 6 * H], F16, kind="ExternalOutput")
        dbg["EA"] = nc.dram_tensor("dbg_EA", [128, 3 * H], F16, kind="ExternalOutput")
        dbg["ebc"] = nc.dram_tensor("dbg_ebc", [128, BPC * 2], F32, kind="ExternalOutput")
        dbg["combi2"] = nc.dram_tensor("dbg_combi2", [128, CW], I16, kind="ExternalOutput")
        dbg["gi0"] = nc.dram_tensor("dbg_gi0", [128, GW], F32, kind="ExternalOutput")
        dbg["gi1"] = nc.dram_tensor("dbg_gi1", [128, GW], F32, kind="ExternalOutput")
        dbg["gv0"] = nc.dram_tensor("dbg_gv0", [128, GW], F32, kind="ExternalOutput")

    with tile.TileContext(nc) as tc:
        with (
            tc.tile_pool(name="const", bufs=1) as cpool,
            tc.tile_pool(name="tables", bufs=1) as tpool,
            tc.tile_pool(name="routing", bufs=1) as rpool,
            tc.tile_pool(name="gath", bufs=2) as gpool,
            tc.tile_pool(name="mask", bufs=2) as mpool,
            tc.tile_pool(name="s1", bufs=2) as s1pool,
            tc.tile_pool(name="fold", bufs=2) as fpool,
            tc.tile_pool(name="outp", bufs=4) as opool,
            tc.tile_pool(name="body", bufs=2) as bdpool,
            tc.tile_pool(name="psB", bufs=1, space="PSUM") as psB,
            tc.tile_pool(name="psS1", bufs=2, space="PSUM") as psS1,
            tc.tile_pool(name="psS2", bufs=5, space="PSUM") as psS2,
        ):
            ident = cpool.tile([128, 128], F32)
            make_identity(nc, ident[:])
            ones1 = cpool.tile([1, 128], F32)
            nc.vector.memset(ones1[:], 1.0)
            ones14 = cpool.tile([1, BPC], F32)
            nc.vector.memset(ones14[:], 1.0)
            io24 = cpool.tile([128, 1], I32)
            nc.gpsimd.iota(io24[:], pattern=[[0, 1]], base=0, channel_multiplier=GW)
            io24f = cpool.tile([128, 1], F32)
            nc.vector.tensor_copy(io24f[:], io24[:])

            br_sb = rpool.tile([1, E], F32)
            nc.sync.dma_start(out=br_sb[:], in_=br[None, :])
            bases_sb = rpool.tile([E, 1], F32)
            nc.sync.dma_start(out=bases_sb[:], in_=bases[:])
            jJ1 = cpool.tile([128, 128], F32R)
            nc.sync.dma_start(out=jJ1[:], in_=jm1[:])
            jJ2 = cpool.tile([128, 128], F32R)
            nc.sync.dma_start(out=jJ2[:], in_=jm2[:])

            # ---- router: logits = cls4 @ Wr.T + br ----
            comb = rpool.tile([E + BPC, N], F32)
            nc.sync.dma_start(out=comb[0:BPC, :], in_=cls4[:])
            nc.sync.dma_start(out=comb[BPC : BPC + E, :], in_=wr[:])
            NBLK = N // 128
            combt = rpool.tile([128, NBLK * (E + BPC)], F32)
            for j in range(NBLK):
                tp = psB.tile([128, E + BPC], F32, tag="small")
                nc.tensor.transpose(
                    tp[:],
                    comb[0 : E + BPC, 128 * j : 128 * (j + 1)],
                    ident[0 : E + BPC, 0 : E + BPC],
                )
                nc.scalar.copy(combt[:, (E + BPC) * j : (E + BPC) * (j + 1)], tp[:])
            lg_ps = psB.tile([BPC, E], F32, tag="small")
            for j in range(NBLK):
                base = (E + BPC) * j
                nc.tensor.matmul(
                    lg_ps[:],
                    lhsT=combt[:, base : base + BPC],
                    rhs=combt[:, base + BPC : base + BPC + E],
                    start=(j == 0),
                    stop=False,
                )
            nc.tensor.matmul(
                lg_ps[:], lhsT=ones14[:], rhs=br_sb[:], start=False, stop=True
            )
            logits = rpool.tile([BPC, E], F32)
            nc.vector.tensor_copy(logits[:], lg_ps[:])

            # ---- top-2, renormalized weights, one-hot selectors ----
            max8 = rpool.tile([BPC, 8], F32)
            nc.vector.max(out=max8[:], in_=logits[:])
            l0 = max8[:, 0:1]
            l1 = max8[:, 1:2]
            d = rpool.tile([BPC, 1], F32)
            nc.vector.tensor_sub(d[:], l1, l0)  # l1 - l0
            dT_ps = psB.tile([1, BPC], F32, tag="small")
            nc.tensor.transpose(dT_ps[:], d[:], ident[0:BPC, 0:BPC])
            dT = rpool.tile([1, BPC], F32)
            nc.vector.tensor_copy(dT[:], dT_ps[:])
            w1T = rpool.tile([1, BPC], F32)
            nc.scalar.activation(w1T[:], dT[:], mybir.ActivationFunctionType.Sigmoid)
            w0T = rpool.tile([1, BPC], F32)
            nc.scalar.activation(
                w0T[:], dT[:], mybir.ActivationFunctionType.Sigmoid, scale=-1.0
            )
            oh1 = rpool.tile([BPC, E], F32)
            oh2 = rpool.tile([BPC, E], F32)
            nc.vector.tensor_scalar(oh1[:], logits[:], l0, None, op0=AOT.is_equal)
            nc.vector.tensor_scalar(oh2[:], logits[:], l1, None, op0=AOT.is_equal)
            selT = []
            for srcap in (oh1, oh2):
                sp = psB.tile([E, BPC], F32, tag="small")
                nc.tensor.transpose(sp[:], srcap[:], ident[0:BPC, 0:BPC])
                sbt = rpool.tile([E, BPC], F32, tag=f"selT{len(selT)}")
                nc.vector.tensor_copy(sbt[:], sp[:])
                selT.append(sbt)
            o1T, o2T = selT

            # per-sample scalar rows [1, BPC]: expert table offsets
            eT = []
            for oT in (o1T, o2T):
                ep = psB.tile([1, BPC], F32, tag="small")
                nc.tensor.matmul(
                    ep[:], lhsT=bases_sb[:], rhs=oT[:], start=True, stop=True
                )
                es = rpool.tile([1, BPC], F32, tag=f"eT{len(eT)}")
                nc.vector.tensor_copy(es[:], ep[:])
                eT.append(es)

            # broadcast each scalar row to all 128 partitions: [128, BPC]
            bc = []
            for rowap in (eT[0], eT[1], w0T, w1T):
                bp = psB.tile([128, BPC], F32, tag="small")
                nc.tensor.matmul(
                    bp[:], lhsT=ones1[:], rhs=rowap[:], start=True, stop=True
                )
                bs = rpool.tile([128, BPC], F32, tag=f"bc{len(bc)}")
                nc.vector.tensor_copy(bs[:], bp[:])
                bc.append(bs)
            ebc = bc[0:2]    # expert base offsets per slot
            wbc = bc[2:4]    # expert weights per slot

            # ---- big table loads AFTER the routing-phase DMAs ----
            rhse_sb = tpool.tile([128, 3 * H], F16, tag="rhse")
            rhso_sb = tpool.tile([128, 3 * H], F16, tag="rhso")
            cx_sb = {}
            stg_pairs = [(rhse, rhse_sb), (rhso, rhso_sb)]
            for nm, t in (("cA", cxa), ("cB", cxb), ("sA", sxa), ("sB", sxb)):
                ts = tpool.tile([128, 3 * 256], F16, tag=nm)
                cx_sb[nm] = ts
                stg_pairs.append((t, ts))
            for k, (t, ts) in enumerate(stg_pairs):
                stg = tpool.tile([128, 3 * H], F32, tag=f"stg{k % 2}")
                w = t.shape[1]
                nc.sync.dma_start(out=stg[:, 0:w], in_=t[:])
                nc.vector.tensor_copy(ts[:], stg[:, 0:w])

            for b in range(BPC):
                # ---- gather entry lists for both slots, interleave per chunk ----
                comb_i = gpool.tile([128, CW], I16, tag="ci")
                comb_v = gpool.tile([128, CW], F16, tag="cv")
                for slot in range(2):
                    offf = gpool.tile([128, 1], F32, tag="offf")
                    nc.vector.tensor_add(offf[:], ebc[slot][:, b : b + 1], io24f[:])
                    offs = gpool.tile([128, 1], I32, tag="offs")
                    nc.vector.tensor_copy(offs[:], offf[:])
                    gi = gpool.tile([128, GW], F32, tag=f"gi{slot}")
                    gv = gpool.tile([128, GW], F32, tag=f"gv{slot}")
                    nc.gpsimd.indirect_dma_start(
                        out=gi[:], out_offset=None, in_=uidx[:],
                        in_offset=IndirectOffsetOnAxis(ap=offs[:], axis=1),
                    )
                    nc.gpsimd.indirect_dma_start(
                        out=gv[:], out_offset=None, in_=cval[:],
                        in_offset=IndirectOffsetOnAxis(ap=offs[:], axis=1),
                    )
                    # interleaved views: slot s occupies cols [j*2NI + s*NI, +NI)
                    vi = comb_i[:].rearrange("p (j r) -> p j r", r=2 * NI)[
                        :, :, slot * NI : (slot + 1) * NI
                    ]
                    vv = comb_v[:].rearrange("p (j r) -> p j r", r=2 * NI)[
                        :, :, slot * NI : (slot + 1) * NI
                    ]
                    if DEBUG and b == DBG_B:
                        nc.sync.dma_start(out=dbg[f"gi{slot}"][:], in_=gi[:])
                        if slot == 0:
                            nc.sync.dma_start(out=dbg["gv0"][:], in_=gv[:])
                    gi3 = gi[:].rearrange("p (j ni) -> p j ni", ni=NI)
                    gv3 = gv[:].rearrange("p (j ni) -> p j ni", ni=NI)
                    if CONTIG_TEST:
                        nc.vector.tensor_copy(
                            comb_i[:, slot * GW : (slot + 1) * GW], gi[:]
                        )
                        nc.vector.tensor_scalar(
                            comb_v[:, slot * GW : (slot + 1) * GW], gv[:],
                            wbc[slot][:, b : b + 1], None, op0=AOT.mult,
                        )
                    else:
                        nc.vector.tensor_copy(vi, gi3)
                        nc.vector.tensor_scalar(
                            vv, gv3, wbc[slot][:, b : b + 1], None, op0=AOT.mult
                        )

                if DEBUG and b == DBG_B:
                    nc.sync.dma_start(out=dbg["combi2"][:], in_=comb_i[:])
                if STOP_EARLY:
                    continue
                # ---- scatter mask chunks MT[v,u] (fp16), then parity fold ----
                mtU = mpool.tile([128, 6 * N], F16, tag="mtU")
                for j in range(6):
                    nc.gpsimd.local_scatter(
                        out_ap=mtU[:, N * j : N * (j + 1)],
                        data_ap=comb_v[:, 2 * NI * j : 2 * NI * (j + 1)],
                        idxs_ap=comb_i[:, 2 * NI * j : 2 * NI * (j + 1)],
                        channels=128, num_elems=N, num_idxs=2 * NI,
                    )
                if DEBUG and b == DBG_B:
                    nc.sync.dma_start(out=dbg["mtU"][:], in_=mtU[:])
                    nc.sync.dma_start(out=dbg["combi"][:], in_=comb_i[:])
                    nc.sync.dma_start(out=dbg["combv"][:], in_=comb_v[:])
                    nc.sync.dma_start(out=dbg["ebc"][:, 0:BPC], in_=ebc[0][:])
                    nc.sync.dma_start(out=dbg["ebc"][:, BPC : 2 * BPC], in_=ebc[1][:])
                mP = mpool.tile([128, 3 * N], F16, tag="mP")
                mM = mpool.tile([128, 3 * N], F16, tag="mM")
                for c in range(3):
                    nc.vector.tensor_tensor(
                        mP[:, N * c : N * (c + 1)],
                        mtU[:, N * c : N * (c + 1)],
                        mtU[:, N * (c + 3) : N * (c + 4)],
                        op=AOT.add,
                    )
                    nc.vector.tensor_tensor(
                        mM[:, N * c : N * (c + 1)],
                        mtU[:, N * c : N * (c + 1)],
                        mtU[:, N * (c + 3) : N * (c + 4)],
                        op=AOT.subtract,
                    )

                # ---- stage 1: e_sb[i] = [aPe | -aQe], o_sb[i] = [aPo | -aQo] ----
                e_sb = s1pool.tile([128, 6 * H], F16, tag="e_sb")
                o_sb = s1pool.tile([128, 6 * H], F16, tag="o_sb")
                for i in range(6):
                    for msrc, rsrc, dst in ((mP, rhse_sb, e_sb), (mM, rhso_sb, o_sb)):
                        acc = psS1.tile([128, H], F32, tag="s1")
                        for c in range(3):
                            nc.tensor.matmul(
                                acc[:],
                                lhsT=msrc[:, N * c + 128 * i : N * c + 128 * (i + 1)],
                                rhs=rsrc[:, H * c : H * (c + 1)],
                                start=(c == 0), stop=(c == 2),
                            )
                        nc.scalar.copy(dst[:, H * i : H * (i + 1)], acc[:])

                if DEBUG and b == DBG_B:
                    nc.sync.dma_start(out=dbg["mP"][:], in_=mP[:])
                    nc.sync.dma_start(out=dbg["esb"][:], in_=e_sb[:])
                    nc.sync.dma_start(out=dbg["osb"][:], in_=o_sb[:])
                # ---- u-parity fold: EA/EB/OA/OB[c] = s1[c] +- s1[c+3] ----
                ftiles = {}
                for nm, src, op in (
                    ("EA", e_sb, AOT.add), ("EB", e_sb, AOT.subtract),
                    ("OA", o_sb, AOT.add), ("OB", o_sb, AOT.subtract),
                ):
                    ft = fpool.tile([128, 3 * H], F16, tag=nm)
                    for c in range(3):
                        nc.vector.tensor_tensor(
                            ft[:, H * c : H * (c + 1)],
                            src[:, H * c : H * (c + 1)],
                            src[:, H * (c + 3) : H * (c + 4)],
                            op=op,
                        )
                    ftiles[nm] = ft

                if DEBUG and b == DBG_B:
                    nc.sync.dma_start(out=dbg["EA"][:], in_=ftiles["EA"][:])
                # ---- stage 2 + assembly ----
                for pi, beta in (("A", 0), ("B", 0), ("A", 1), ("B", 1)):
                    t1s = {}
                    t2p = {}
                    for g in ("E", "O"):
                        src = ftiles[g + pi]
                        p1 = psS2.tile([128, HP], F32, tag="s2")
                        p2 = psS2.tile([128, HP], F32, tag="s2")
                        for c in range(3):
                            base = 256 * c + 128 * beta
                            nc.tensor.matmul(
                                p1[:],
                                lhsT=cx_sb["c" + pi][:, base : base + 128],
                                rhs=src[:, H * c : H * c + HP],
                                start=(c == 0), stop=(c == 2),
                            )
                        for c in range(3):
                            base = 256 * c + 128 * beta
                            nc.tensor.matmul(
                                p2[:],
                                lhsT=cx_sb["s" + pi][:, base : base + 128],
                                rhs=src[:, H * c + HP : H * (c + 1)],
                                start=(c == 0), stop=(c == 2),
                            )
                        # T1 half to SBUF (tensor_tensor may read only one PSUM ap)
                        ts = opool.tile([128, HP], F32, tag="t1s")
                        nc.scalar.copy(ts[:], p1[:])
                        t1s[g] = ts
                        t2p[g] = p2

                    # direct tile: rows x = 2m + (pi == B), m in [128b, 128(b+1))
                    pe1, pe2 = t1s["E"], t2p["E"]
                    po1, po2 = t1s["O"], t2p["O"]
                    ob = opool.tile([128, N], F32, tag="ob")
                    nc.vector.tensor_tensor(
                        ob[:, 0:386:2], pe1[:], pe2[:], op=AOT.add
                    )
                    nc.vector.tensor_tensor(
                        ob[:][:, 766:384:-2], pe1[:, 1:192], pe2[:, 1:192],
                        op=AOT.subtract,
                    )
                    nc.vector.tensor_tensor(
                        ob[:, 1:386:2], po1[:], po2[:], op=AOT.add
                    )
                    nc.vector.tensor_tensor(
                        ob[:][:, 767:385:-2], po1[:, 0:191], po2[:, 0:191],
                        op=AOT.subtract,
                    )
                    row0 = 2 * 128 * beta + (1 if pi == "B" else 0)
                    nc.scalar.dma_start(
                        out=out4[:][b][row0 : row0 + 256 : 2, :], in_=ob[:]
                    )

                    # mirror bodies: sum/dif swapped, from (A,0),(B,0) + (A,1) row0
                    def swapped_into(dst, sl):
                        nc.vector.tensor_tensor(
                            dst[:, 0:386:2][sl], pe1[:][sl], pe2[:][sl],
                            op=AOT.subtract,
                        )
                        nc.vector.tensor_tensor(
                            dst[:][:, 766:384:-2][sl], pe1[:, 1:192][sl],
                            pe2[:, 1:192][sl], op=AOT.add,
                        )
                        nc.vector.tensor_tensor(
                            dst[:, 1:386:2][sl], po1[:][sl], po2[:][sl],
                            op=AOT.subtract,
                        )
                        nc.vector.tensor_tensor(
                            dst[:][:, 767:385:-2][sl], po1[:, 0:191][sl],
                            po2[:, 0:191][sl], op=AOT.add,
                        )

                    if pi == "A" and beta == 0:
                        bodyE = bdpool.tile([128, N], F32R, tag="bodyE")
                        swapped_into(bodyE, np.s_[:, :])
                    elif pi == "B" and beta == 0:
                        bodyO = bdpool.tile([128, N], F32R, tag="bodyO")
                        swapped_into(bodyO, np.s_[:, :])
                    elif pi == "A" and beta == 1:
                        swapped_into(bodyE, np.s_[0:1, :])

                # rotate bodies into mirror rows and emit
                for body, jj, row0 in ((bodyE, jJ1, 512), (bodyO, jJ2, 513)):
                    obm = opool.tile([128, N], F32, tag="obm")
                    for c0 in (0, 384):
                        mm = psS1.tile([128, H], F32, tag="s1")
                        nc.tensor.matmul(
                            mm[:, 0:384],
                            lhsT=jj[:], rhs=body[:, c0 : c0 + 384],
                            start=True, stop=True,
                        )
                        nc.scalar.copy(obm[:, c0 : c0 + 384], mm[:, 0:384])
                    nc.scalar.dma_start(
                        out=out4[:][b][row0 : N : 2, :], in_=obm[:]
                    )

    nc.compile()
    return nc


def _get_nc():
    global _NC
    if _NC is None:
        _NC = _build()
    return _NC


def _host_entry_tables(list_indices, coeff):
    """Per (expert, v): up to NI (u, coeff) entries, -1/0 padded; DRAM layout
    [E, 128 partitions, 6 v-chunks, NI] so one indirect-DMA run per partition
    covers all six chunks."""
    li = list_indices.astype(np.int64)
    uu = li // N
    vv = li % N
    ui = np.full((E, N, NI), -1.0, np.float32)
    cv = np.zeros((E, N, NI), np.float32)
    for e in range(E):
        order = np.argsort(vv[e], kind="stable")
        vs = vv[e][order]
        starts = np.searchsorted(vs, np.arange(N))
        ranks = np.arange(NF) - starts[vs]
        assert ranks.max() < NI, f"NI overflow: {ranks.max() + 1}"
        ui[e, vs, ranks] = uu[e][order].astype(np.float32)
        cv[e, vs, ranks] = coeff[e][order].astype(np.float32)
    ui = ui.reshape(E, 6, 128, NI).transpose(0, 2, 1, 3).reshape(E, -1)
    cv = cv.reshape(E, 6, 128, NI).transpose(0, 2, 1, 3).reshape(E, -1)
    return np.ascontiguousarray(ui), np.ascontiguousarray(cv)


def _host_cs_tables():
    a = np.arange(N)
    ang = 2.0 * np.pi * ((a[:, None] * a[None, :]) % N) / N
    cosT = np.cos(ang)
    sinT = np.sin(ang)
    sc = ALPHA / (N * N)
    ye = np.arange(0, H, 2)
    yo = np.arange(1, H, 2)
    rhse_v = np.zeros((128, 3 * H), np.float32)
    rhso_v = np.zeros((128, 3 * H), np.float32)
    for c in range(3):
        rows = slice(128 * c, 128 * (c + 1))
        rhse_v[:, H * c : H * c + HP] = sc * cosT[rows][:, ye]
        rhse_v[:, H * c + HP : H * (c + 1)] = -sc * sinT[rows][:, ye]
        rhso_v[:, H * c : H * c + HP] = sc * cosT[rows][:, yo]
        rhso_v[:, H * c + HP : H * (c + 1)] = -sc * sinT[rows][:, yo]
    xsA = np.arange(0, 512, 2)
    xsB = np.arange(1, 512, 2)
    packs = {}
    for nm, tab, xs in (
        ("cxa", cosT, xsA), ("cxb", cosT, xsB),
        ("sxa", sinT, xsA), ("sxb", sinT, xsB),
    ):
        pk = np.zeros((128, 3 * 256), np.float32)
        for c in range(3):
            for beta in range(2):
                pk[:, 256 * c + 128 * beta : 256 * c + 128 * (beta + 1)] = (
                    tab[128 * c : 128 * (c + 1)][:, xs[128 * beta : 128 * (beta + 1)]]
                )
        packs[nm] = pk
    return rhse_v, rhso_v, packs


def kernel(cls_token, W_router, b_router, coeff, list_indices):
    global LAST_RESULT
    cls_token = np.asarray(cls_token)
    W_router = np.asarray(W_router)
    b_router = np.asarray(b_router)
    coeff = np.asarray(coeff)
    list_indices = np.asarray(list_indices)
    assert cls_token.shape == (B, N) and coeff.shape == (E, NF)
    nc = _get_nc()
    uiv, cvv = _host_entry_tables(list_indices, coeff)
    rhse_v, rhso_v, packs = _host_cs_tables()
    basesv = (np.arange(E, dtype=np.float32) * (128 * GW)).reshape(E, 1)
    jm1v = np.zeros((128, 128), np.float32)
    for m_ in range(128):
        jm1v[(128 - m_) % 128, m_] = 1.0
    jm2v = np.zeros((128, 128), np.float32)
    for m_ in range(128):
        jm2v[127 - m_, m_] = 1.0
    wrr = np.ascontiguousarray(W_router, dtype=np.float32)
    brr = np.ascontiguousarray(b_router, dtype=np.float32)
    in_maps = []
    for c in range(NCORES):
        in_maps.append(
            {
                "cls4": np.ascontiguousarray(
                    cls_token[BPC * c : BPC * (c + 1)], dtype=np.float32
                ),
                "wr": wrr,
                "br": brr,
                "uidx": uiv,
                "cval": cvv,
                "bases": basesv,
                "jm1": jm1v,
                "jm2": jm2v,
                "rhse": rhse_v,
                "rhso": rhso_v,
                "cxa": packs["cxa"],
                "cxb": packs["cxb"],
                "sxa": packs["sxa"],
                "sxb": packs["sxb"],
            }
        )
    res = run_bass_kernel_spmd(
        nc, in_maps, core_ids=list(range(NCORES)), trace=KERNEL_TRACE
    )
    LAST_RESULT = res
    out = np.concatenate([res.results[c]["out4"] for c in range(NCORES)], axis=0)
    return out
